# revision 37
# baseline (speedup 1.0000x reference)
"""Trainium2 Bass kernel for NeuronGemma4VisionAttention.

Problem: B=2, P=4096, HID=1152, 16 heads x 72 dim, fp32.
  q,k,v = x@Wq, x@Wk, x@Wv  -> per-head RMSNorm (q,k learned scale, v none)
  -> 2-part RoPE on q,k -> softmax(q k^T) v -> concat heads @ Wo

Sharding (8 cores, one chip):
  Head-parallel: core c owns heads (2c, 2c+1) for BOTH batches.
  Each core: QKV projection (its 144 columns of each W), per-head norm+rope,
  full non-causal attention for its 2 heads x 2 batches; two half-token
  AllToAlls exchange token-eighths (second one overlapped with o_proj of the
  first half) so core c ends with the full 1152-dim attention output for
  tokens [1024*(c%4) ...) of batch c//4, on which it runs the o_proj.

Key implementation notes (v3):
  - Scalar engine runs ONLY the softmax Exp (plus a few copies): RMSNorm
    rsqrt is a bit-trick+Newton on DVE/Pool, the softmax stabilizer c_q uses
    a seed-only sqrt (c cancels in softmax; only its range matters --
    validated max(rowmax-c)=32<80).
  - Exp in [128,512] single-PSUM-bank tiles (bank-crossing reads are slower).
  - P (exp scores), V, A2A payload, o_proj in bf16 (e2e 3.1e-3); q/k/scores
    stay f32r (bf16 there would be 1.2-1.5e-2, too close to the 2e-2 gate).
  - Softmax denominators via the accurate DVE reciprocal macro: they span
    down to 1e-27 (c overshoots rowmax by up to ~60), where
    reciprocal_approx_fast returns garbage.
  - DMA: host pre-blocks x/ropec so descriptors are >=2.3KB; x loads in
    2-block groups on the SP queue, ropec on the Act queue, qt
    staging written in 4-block groups and all-to-all payload on the Pool
    (SWDGE) queue -- one HWDGE queue was the phase-1 bottleneck before.
  - Phase1(b=1) emission interleaved with phase2(b=0); pso double-buffered
    so the normalize/reciprocal chain is off the PE critical path.
"""
import os
import sys

sys.path.insert(0, "/opt/trn_rl_repo")

import numpy as np
import ml_dtypes

import concourse.bass as bass  # noqa: F401
import concourse.tile as tile
from concourse import bacc, mybir
from concourse.bass_utils import run_bass_kernel_spmd
from concourse.masks import make_identity

F32 = mybir.dt.float32
F32R = mybir.dt.float32r
BF16 = mybir.dt.bfloat16
I32 = mybir.dt.int32
AF = mybir.ActivationFunctionType
ALU = mybir.AluOpType

N_CORES = 8
B, P, HID = 2, 4096, 1152
NH, D = 16, 72
HL = 2                 # heads per core
TB = B * P
NBLK = P // 128        # 32 token blocks per batch
QC = 512               # query chunk
KBLK = P // 128        # 32 key blocks per batch
SQ72 = 8.48528137423857   # sqrt(72)
BETA = 8.0
MAGIC = 0x5F3759DF

_CACHED_NC = None


def _emit_ph1_pair(nc, ctx, b, kbp):
    """Two 128-token blocks: loads grouped, compute per block."""
    p = ctx
    gpair = b * NBLK + 2 * kbp
    xt = p["p1"].tile([128, 2, 9, 128], F32R, tag="xt", name="xt")
    nc.sync.dma_start(xt[:], p["xb"][:, gpair:gpair + 2])
    rc2 = p["p1"].tile([128, 2, 4, D], F32, tag="rc", name="rc2")
    nc.gpsimd.dma_start(rc2[:], p["rcb"][:, gpair:gpair + 2])

    for half in range(2):
        kb = 2 * kbp + half
        _emit_ph1_block(nc, ctx, b, kb, xt[:, half], rc2[:, half])


def _emit_ph1_block(nc, ctx, b, kb, xt, rc):
    p = ctx

    psq = p["p1ps"].tile([128, 3 * HL * D], F32, tag="psq", name="psq")
    for c in range(9):
        nc.tensor.matmul(psq[:], xt[:, c, :], p["wqkv_sb"][:, c, :],
                         start=(c == 0), stop=(c == 8))

    sb = p["p1"].tile([128, 3 * HL * D], F32, tag="sb", name="sb")
    nc.scalar.activation(sb[:], psq[:], AF.Copy)

    # ssr[g] = sum_d qkv[g,d]^2 (6 fused square+reduce)
    ssr = p["p1"].tile([128, 6], F32, tag="ssr", name="ssr")
    sqd = p["p1"].tile([128, D], F32, tag="sqd", name="sqd")
    for g in range(6):
        gs = sb[:, g * D:(g + 1) * D]
        nc.vector.scalar_tensor_tensor(sqd[:], gs, 1.0, gs,
                                       op0=ALU.mult, op1=ALU.mult,
                                       accum_out=ssr[:, g:g + 1])

    # y = rsqrt(ssr) via bit-trick seed + 2 Newton iterations.  Pool only
    # has tensor-tensor mult/add, so precompute hneg=-ssr/2 on DVE and use
    # the 1.5-constant tile: y <- y * (c15 + (y*y)*hneg).
    y = p["p1"].tile([128, 6], F32, tag="y", name="y")
    t1 = p["p1"].tile([128, 6], F32, tag="t1", name="t1")
    hneg = p["p1"].tile([128, 6], F32, tag="hneg", name="hneg")
    nc.vector.tensor_scalar(y[:].bitcast(I32), ssr[:].bitcast(I32),
                            1, None, op0=ALU.logical_shift_right)
    nc.vector.tensor_tensor(y[:].bitcast(I32), p["magic"][:, 0:6],
                            y[:].bitcast(I32), op=ALU.subtract)
    nc.vector.tensor_scalar_mul(hneg[:], ssr[:], -0.5)
    for _ in range(2):
        nc.vector.tensor_mul(t1[:], y[:], y[:])
        nc.vector.tensor_mul(t1[:], t1[:], hneg[:])
        nc.vector.tensor_add(t1[:], t1[:], p["c15"][:, 0:6])
        nc.vector.tensor_mul(y[:], y[:], t1[:])
    nc.vector.tensor_scalar_mul(y[:], y[:], SQ72)

    # rope on q (groups 0:2) and k (groups 2:4): expand per-t cos/sin rows
    # across hl heads (3D broadcast copies), then 6 elementwise ops.
    rc3 = rc.rearrange("p (t s) d -> p t s d", s=2)
    cwx = p["p1"].tile([128, 2, HL, D], F32, tag="cwx", name="cwx")
    swx = p["p1"].tile([128, 2, HL, D], F32, tag="swx", name="swx")
    nc.vector.tensor_copy(
        cwx[:], rc3[:, :, 0, :].unsqueeze(2).to_broadcast([128, 2, HL, D]))
    nc.vector.tensor_copy(
        swx[:], rc3[:, :, 1, :].unsqueeze(2).to_broadcast([128, 2, HL, D]))
    qk5 = sb[:, 0:2 * HL * D].rearrange("p (g a c j) -> p g a c j",
                                        g=2 * HL, a=2, c=2)
    cw5 = cwx[:].rearrange("p t hl (a c j) -> p (t hl) a c j", a=2, c=2)
    sw5 = swx[:].rearrange("p t hl (a c j) -> p (t hl) a c j", a=2, c=2)
    rp = p["p1"].tile([128, 2 * HL, D], F32, tag="rp", name="rp")
    rp5 = rp[:].rearrange("p g (a c j) -> p g a c j", a=2, c=2)
    tmp = p["p1"].tile([128, 2 * HL, 2, 18], F32, tag="rtmp", name="rtmp")
    nc.vector.tensor_mul(rp5[:, :, :, 0, :], qk5[:, :, :, 0, :],
                         cw5[:, :, :, 0, :])
    nc.vector.tensor_mul(tmp[:], qk5[:, :, :, 1, :], sw5[:, :, :, 0, :])
    nc.vector.tensor_sub(rp5[:, :, :, 0, :], rp5[:, :, :, 0, :], tmp[:])
    nc.vector.tensor_mul(rp5[:, :, :, 1, :], qk5[:, :, :, 1, :],
                         cw5[:, :, :, 1, :])
    nc.vector.tensor_mul(tmp[:], qk5[:, :, :, 0, :], sw5[:, :, :, 1, :])
    nc.vector.tensor_add(rp5[:, :, :, 1, :], rp5[:, :, :, 1, :], tmp[:])

    # ss2[hl] = sum_d rope(q_raw)^2
    ss2 = p["p1"].tile([128, HL], F32, tag="ss2", name="ss2")
    for hl in range(HL):
        rs = rp[:, hl, :]
        nc.vector.scalar_tensor_tensor(sqd[:], rs, 1.0, rs,
                                       op0=ALU.mult, op1=ALU.mult,
                                       accum_out=ss2[:, hl:hl + 1])

    # c_q = 8*sqrt(72)*y_q*(ss2*seed(ss2)); c's precision is irrelevant
    # (cancels in softmax), only its range matters.
    y0s = p["p1"].tile([128, HL], F32, tag="y0s", name="y0s")
    m1 = p["p1"].tile([128, HL], F32, tag="m1", name="m1")
    nc.vector.tensor_scalar(y0s[:].bitcast(I32), ss2[:].bitcast(I32),
                            1, None, op0=ALU.logical_shift_right)
    nc.vector.tensor_tensor(y0s[:].bitcast(I32), p["magic"][:, 0:HL],
                            y0s[:].bitcast(I32), op=ALU.subtract)
    nc.vector.tensor_mul(m1[:], ss2[:], y0s[:])
    nc.vector.tensor_mul(m1[:], m1[:], y[:, 0:2])

    qaug = p["p1"].tile([128, HL, D + 1], F32, tag="qaug", name="qaug")
    kaug = p["p1"].tile([128, HL, D + 1], F32, tag="kaug", name="kaug")
    nc.vector.tensor_mul(qaug[:, :, D], m1[:], p["cneg"][:, 0:HL])
    nc.vector.memset(kaug[:, :, D], 1.0)

    ybq = y[:, 0:2].unsqueeze(2).to_broadcast([128, HL, D])
    ybk = y[:, 2:4].unsqueeze(2).to_broadcast([128, HL, D])
    ybv = y[:, 4:6].unsqueeze(2).to_broadcast([128, HL, D])
    nc.vector.tensor_mul(qaug[:, :, 0:D], rp[:, 0:2, :], ybq)
    nc.vector.tensor_mul(kaug[:, :, 0:D], rp[:, 2:4, :], ybk)
    nc.vector.tensor_mul(
        p["vaug"][b][:, kb, :, 0:D],
        sb[:, 2 * HL * D:3 * HL * D].rearrange("p (hl d) -> p hl d", hl=HL),
        ybv)

    # transpose q/k to feature-major; stage q for 4 blocks per DMA
    tr4 = p["trps"].tile([D + 1, 4, 128], F32, tag="tr4", name="tr4")
    for hl in range(HL):
        nc.tensor.transpose(tr4[:, hl, :], qaug[:, hl, :], p["ident"][:])
        nc.tensor.transpose(tr4[:, 2 + hl, :], kaug[:, hl, :], p["ident"][:])
    if kb % 4 == 0:
        ctx["qstg"] = p["p1"].tile([D + 1, HL, 512], F32R, tag="qstg",
                                   name="qstg")
    qs = kb % 4
    nc.vector.tensor_copy(ctx["qstg"][:, :, qs * 128:(qs + 1) * 128],
                          tr4[:, 0:2, :])
    nc.vector.tensor_copy(p["kt"][b][:, :, kb * 128:(kb + 1) * 128],
                          tr4[:, 2:4, :])
    if kb % 4 == 3:
        k4 = kb - 3
        nc.gpsimd.dma_start(
            p["qt_dram"][b][:, :, k4 * 128:(k4 + 4) * 128], ctx["qstg"][:])


def _emit_normalize(nc, ctx, src_ap, b, hl, qc):
    p = ctx
    rec = p["p2n"].tile([1, QC], F32, tag="rec", name="rec")
    nc.vector.reciprocal(rec[:], src_ap[96:97, :])
    bct = p["p2n"].tile([D, QC], F32, tag="bct", name="bct")
    nc.gpsimd.partition_broadcast(bct[:], rec[:])
    onrm = p["p2n"].tile([D, QC], BF16, tag="onrm", name="onrm")
    nc.vector.tensor_mul(onrm[:], src_ap[0:D, :], bct[:])
    e = b * 4 + qc // 2
    h = qc % 2
    nc.gpsimd.dma_start(p["a2a_in"][h][e, hl * D:(hl + 1) * D, :], onrm[:])


def _emit_ph2_iter(nc, ctx, b, hl, qc, defer=False):
    """Full attention for one (batch, head, 512-query chunk)."""
    p = ctx
    qsl = slice(qc * QC, (qc + 1) * QC)
    qt_t = p["p2"].tile([D + 1, QC], F32R, tag="qt", name="qt_t")
    nc.sync.dma_start(qt_t[:], p["qt_dram"][b][:, hl, qsl])

    pso = p["psos"].tile([97, QC], F32, tag="pso", name="pso")
    for kb in range(KBLK):
        ksl = slice(kb * 128, (kb + 1) * 128)
        pss = p["pss"].tile([128, QC], F32, tag="pss", name="pss")
        nc.tensor.matmul(pss[:], p["kt"][b][:, hl, ksl], qt_t[:],
                         start=True, stop=True)
        pt = p["p2"].tile([128, QC], BF16, tag="pt", name="pt")
        nc.scalar.activation(pt[:], pss[:], AF.Exp)
        nc.tensor.matmul(pso[:], p["vaug"][b][:, kb, hl, :], pt[:],
                         start=(kb == 0), stop=(kb == KBLK - 1))

    if defer:
        # stash the raw accumulator; normalize later when DVE is idle
        # (the even-half outputs aren't needed until the first AllToAll)
        st = p["stash"].tile([97, QC], BF16, tag=f"st_{b}_{hl}_{qc}",
                             name=f"st_{b}_{hl}_{qc}")
        nc.vector.tensor_copy(st[:], pso[:])
        p["deferred"].append((st, b, hl, qc))
    else:
        _emit_normalize(nc, ctx, pso[:], b, hl, qc)


def _emit_oproj_half(nc, ctx, tcn):
    p = ctx
    tsl = slice(tcn * QC, (tcn + 1) * QC)
    yt = p["p3"].tile([128, 9, QC], BF16, tag=f"yt{tcn}", name=f"yt{tcn}")
    nc.sync.dma_start(
        yt[:],
        p["a2a_out"][tcn][:].rearrange("j r t -> (j r) t").rearrange(
            "(c p) t -> p c t", p=128))
    for fo in range(9):
        ps3 = p["p3ps"].tile([128, QC], F32, tag="ps3", name="ps3")
        for fi in range(9):
            nc.tensor.matmul(ps3[:],
                             p["wo_sb"][:, fi, fo * 128:(fo + 1) * 128],
                             yt[:, fi, :], start=(fi == 0), stop=(fi == 8))
        ot = p["p3o"].tile([128, QC], F32, tag="ot", name="ot")
        nc.scalar.activation(ot[:], ps3[:], AF.Copy)
        nc.sync.dma_start(p["outT"][:, fo, tsl], ot[:])


def _build_nc():
    nc = bacc.Bacc("TRN2", target_bir_lowering=False, debug=False,
                   num_devices=N_CORES)

    xb = nc.dram_tensor("xb", [128, B * NBLK, 9, 128], F32R,
                        kind="ExternalInput").ap()
    rcb = nc.dram_tensor("rcb", [128, B * NBLK, 4, D], F32,
                         kind="ExternalInput").ap()
    wqkv = nc.dram_tensor("wqkv", [128, 9, 3 * HL * D], F32R,
                          kind="ExternalInput").ap()
    wo = nc.dram_tensor("wo", [128, 9, HID], BF16, kind="ExternalInput").ap()
    outT = nc.dram_tensor("outT", [128, 9, 1024], F32,
                          kind="ExternalOutput").ap()

    with tile.TileContext(nc) as tc:
        with (
            tc.tile_pool(name="persist", bufs=1) as persist,
            tc.tile_pool(name="dram", bufs=1, space="DRAM") as dram,
        ):
            ctx = {"xb": xb, "rcb": rcb, "outT": outT}

            ident = persist.tile([128, 128], F32, tag="ident")
            make_identity(nc, ident)
            ctx["ident"] = ident
            magic = persist.tile([128, 6], I32, tag="magic")
            nc.vector.memset(magic[:], MAGIC)
            ctx["magic"] = magic
            c15 = persist.tile([128, 6], F32, tag="c15")
            nc.vector.memset(c15[:], 1.5)
            ctx["c15"] = c15
            cneg = persist.tile([128, 2], F32, tag="cneg")
            nc.vector.memset(cneg[:], -BETA)
            ctx["cneg"] = cneg

            wqkv_sb = persist.tile([128, 9, 3 * HL * D], F32R, tag="wqkv")
            nc.sync.dma_start(wqkv_sb[:], wqkv)
            ctx["wqkv_sb"] = wqkv_sb

            ctx["kt"] = [persist.tile([D + 1, HL, P], F32R, tag=f"kt{b}",
                                      name=f"kt{b}") for b in range(B)]
            ctx["vaug"] = [persist.tile([128, KBLK, HL, 97], BF16,
                                        tag=f"vaug{b}", name=f"vaug{b}")
                           for b in range(B)]
            for b in range(B):
                nc.vector.memset(ctx["vaug"][b][:], 0.0)
                nc.vector.memset(ctx["vaug"][b][:, :, :, 96], 1.0)

            ctx["qt_dram"] = [dram.tile([D + 1, HL, P], F32R,
                                        name=f"qtd{b}") for b in range(B)]
            ctx["a2a_in"] = [dram.tile([N_CORES, HL * D, QC], BF16,
                                       name=f"a2ai{h}") for h in range(2)]
            ctx["a2a_out"] = [dram.tile([N_CORES, HL * D, QC], BF16,
                                        name=f"a2ao{h}") for h in range(2)]

            evens = [0, 2, 4, 6]
            odds = [1, 3, 5, 7]

            ctx["deferred"] = []
            with (
                tc.tile_pool(name="stash", bufs=1) as stash,
                tc.tile_pool(name="p2", bufs=4) as p2,
                tc.tile_pool(name="p2n", bufs=2) as p2n,
                tc.tile_pool(name="pss", bufs=4, space="PSUM") as pss,
                tc.tile_pool(name="psos", bufs=2, space="PSUM") as psos,
            ):
                ctx.update(p2=p2, p2n=p2n, pss=pss, psos=psos, stash=stash)

                # ---- phase 1 (b=0), then phase2(b=0 evens) x phase1(b=1)
                with (
                    tc.tile_pool(name="p1", bufs=3) as p1,
                    tc.tile_pool(name="p1ps", bufs=1, space="PSUM") as p1ps,
                    tc.tile_pool(name="trps", bufs=1, space="PSUM") as trps,
                ):
                    ctx.update(p1=p1, p1ps=p1ps, trps=trps)
                    for kbp in range(NBLK // 2):
                        _emit_ph1_pair(nc, ctx, 0, kbp)
                    for j, qc in enumerate(evens):
                        for hl in range(HL):
                            _emit_ph2_iter(nc, ctx, 0, hl, qc)
                            _emit_ph1_pair(nc, ctx, 1, 2 * (2 * j + hl))
                            _emit_ph1_pair(nc, ctx, 1, 2 * (2 * j + hl) + 1)

                for qc in evens:
                    for hl in range(HL):
                        _emit_ph2_iter(nc, ctx, 1, hl, qc)

                nc.gpsimd.collective_compute(
                    "AllToAll", mybir.AluOpType.bypass,
                    ins=[ctx["a2a_in"][0][:]],
                    outs=[ctx["a2a_out"][0][:]],
                    replica_groups=[list(range(N_CORES))],
                )

                with (
                    tc.tile_pool(name="p3", bufs=1) as p3,
                    tc.tile_pool(name="p3o", bufs=2) as p3o,
                    tc.tile_pool(name="p3ps", bufs=2, space="PSUM") as p3ps,
                ):
                    ctx.update(p3=p3, p3o=p3o, p3ps=p3ps)
                    wo_sb = p3.tile([128, 9, HID], BF16, tag="wo",
                                    name="wo_sb")
                    nc.sync.dma_start(wo_sb[:], wo)
                    ctx["wo_sb"] = wo_sb

                    for b in range(B):
                        for qc in odds:
                            for hl in range(HL):
                                _emit_ph2_iter(nc, ctx, b, hl, qc)

                    _emit_oproj_half(nc, ctx, 0)

                    nc.gpsimd.collective_compute(
                        "AllToAll", mybir.AluOpType.bypass,
                        ins=[ctx["a2a_in"][1][:]],
                        outs=[ctx["a2a_out"][1][:]],
                        replica_groups=[list(range(N_CORES))],
                    )

                    _emit_oproj_half(nc, ctx, 1)

    nc.compile()
    return nc


def _prep_inputs(inputs):
    hs = np.asarray(inputs["hidden_states"], dtype=np.float32)
    cos = np.asarray(inputs["cos"], dtype=np.float32)
    sin = np.asarray(inputs["sin"], dtype=np.float32)
    Wq = np.asarray(inputs["Wq"], dtype=np.float32)
    Wk = np.asarray(inputs["Wk"], dtype=np.float32)
    Wv = np.asarray(inputs["Wv"], dtype=np.float32)
    Wo = np.asarray(inputs["Wo"], dtype=np.float32)
    qw = np.asarray(inputs["q_norm_w"], dtype=np.float32)
    kw = np.asarray(inputs["k_norm_w"], dtype=np.float32)

    # x blocked: xb[p, blk, c, ti] = x[t=blk*128+ti, h=c*128+p]
    xb = np.ascontiguousarray(
        hs.reshape(TB // 128, 128, 9, 128).transpose(3, 0, 2, 1))

    # rope coefficients per token: rows [cwq, swq, cwk, swk]
    partner = np.empty(D, np.int64)
    for a in range(2):
        base = a * 36
        partner[base:base + 18] = np.arange(base + 18, base + 36)
        partner[base + 18:base + 36] = np.arange(base, base + 18)
    cs = cos.reshape(TB, D)
    sn = sin.reshape(TB, D)
    ropec = np.stack([cs * qw[None, :], sn * qw[partner][None, :],
                      cs * kw[None, :], sn * kw[partner][None, :]],
                     axis=1).astype(np.float32)
    rcb = np.ascontiguousarray(
        ropec.reshape(TB // 128, 128, 4, D).transpose(1, 0, 2, 3))

    wob = np.ascontiguousarray(
        Wo.reshape(9, 128, HID).transpose(1, 0, 2)).astype(
            ml_dtypes.bfloat16)

    in_maps = []
    for c in range(N_CORES):
        colsl = slice(c * HL * D, (c + 1) * HL * D)
        wqkv = np.concatenate([Wq[:, colsl], Wk[:, colsl], Wv[:, colsl]],
                              axis=1)
        wqkv = np.ascontiguousarray(
            wqkv.reshape(9, 128, 3 * HL * D).transpose(1, 0, 2))
        in_maps.append({
            "xb": xb,
            "rcb": rcb,
            "wqkv": wqkv,
            "wo": wob,
        })
    return in_maps


def kernel(**inputs):
    global _CACHED_NC
    if _CACHED_NC is None:
        _CACHED_NC = _build_nc()
    nc = _CACHED_NC
    in_maps = _prep_inputs(inputs)
    trace = bool(int(os.environ.get("KERNEL_TRACE", "0")))
    res = run_bass_kernel_spmd(nc, in_maps, core_ids=list(range(N_CORES)),
                               trace=trace)
    kernel.last_results = res
    out = np.empty((B, P, HID), dtype=np.float32)
    for c in range(N_CORES):
        b, qtr = c // 4, c % 4
        r = res.results[c]["outT"]  # [128, 9, 1024]
        out[b, qtr * 1024:(qtr + 1) * 1024, :] = \
            np.asarray(r).transpose(2, 1, 0).reshape(1024, HID)
    return out


# revision 38
# speedup vs baseline: 1.0434x; 1.0434x over previous
"""Trainium2 Bass kernel for NeuronGemma4VisionAttention.

Problem: B=2, P=4096, HID=1152, 16 heads x 72 dim, fp32.
  q,k,v = x@Wq, x@Wk, x@Wv  -> per-head RMSNorm (q,k learned scale, v none)
  -> 2-part RoPE on q,k -> softmax(q k^T) v -> concat heads @ Wo

Sharding (8 cores, one chip):
  Head-parallel: core c owns heads (2c, 2c+1) for BOTH batches.
  Each core: QKV projection (its 144 columns of each W), per-head norm+rope,
  full non-causal attention for its 2 heads x 2 batches; two half-token
  AllToAlls exchange token-eighths (second one overlapped with o_proj of the
  first half) so core c ends with the full 1152-dim attention output for
  tokens [1024*(c%4) ...) of batch c//4, on which it runs the o_proj.

Key implementation notes (v3):
  - Scalar engine runs ONLY the softmax Exp (plus a few copies): RMSNorm
    rsqrt is a bit-trick+Newton on DVE/Pool, the softmax stabilizer c_q uses
    a seed-only sqrt (c cancels in softmax; only its range matters --
    validated max(rowmax-c)=32<80).
  - Exp in [128,512] single-PSUM-bank tiles (bank-crossing reads are slower).
  - P (exp scores), V, A2A payload, o_proj in bf16 (e2e 3.1e-3); q/k/scores
    stay f32r (bf16 there would be 1.2-1.5e-2, too close to the 2e-2 gate).
  - Softmax denominators via the accurate DVE reciprocal macro: they span
    down to 1e-27 (c overshoots rowmax by up to ~60), where
    reciprocal_approx_fast returns garbage.
  - DMA: host pre-blocks x/ropec so descriptors are >=2.3KB; x loads in
    2-block groups on the SP queue, ropec on the Act queue, qt
    staging written in 4-block groups and all-to-all payload on the Pool
    (SWDGE) queue -- one HWDGE queue was the phase-1 bottleneck before.
  - Phase1(b=1) emission interleaved with phase2(b=0); pso double-buffered
    so the normalize/reciprocal chain is off the PE critical path.
"""
import os
import sys

sys.path.insert(0, "/opt/trn_rl_repo")

import numpy as np
import ml_dtypes

import concourse.bass as bass  # noqa: F401
import concourse.tile as tile
from concourse import bacc, mybir
from concourse.bass_utils import run_bass_kernel_spmd
from concourse.masks import make_identity

F32 = mybir.dt.float32
F32R = mybir.dt.float32r
BF16 = mybir.dt.bfloat16
I32 = mybir.dt.int32
AF = mybir.ActivationFunctionType
ALU = mybir.AluOpType

N_CORES = 8
B, P, HID = 2, 4096, 1152
NH, D = 16, 72
HL = 2                 # heads per core
TB = B * P
NBLK = P // 128        # 32 token blocks per batch
QC = 512               # query chunk
KBLK = P // 128        # 32 key blocks per batch
SQ72 = 8.48528137423857   # sqrt(72)
BETA = 8.0
MAGIC = 0x5F3759DF

_CACHED_NC = None


def _emit_ph1_pair(nc, ctx, b, kbp):
    """Two 128-token blocks: loads grouped, compute per block."""
    p = ctx
    gpair = b * NBLK + 2 * kbp
    xt = p["p1"].tile([128, 2, 9, 128], F32R, tag="xt", name="xt")
    nc.sync.dma_start(xt[:], p["xb"][:, gpair:gpair + 2])
    rc2 = p["p1"].tile([128, 2, 4, D], F32, tag="rc", name="rc2")
    nc.scalar.dma_start(rc2[:], p["rcb"][:, gpair:gpair + 2])

    for half in range(2):
        kb = 2 * kbp + half
        _emit_ph1_block(nc, ctx, b, kb, xt[:, half], rc2[:, half])


def _emit_ph1_block(nc, ctx, b, kb, xt, rc):
    p = ctx

    psq = p["p1ps"].tile([128, 3 * HL * D], F32, tag="psq", name="psq")
    for c in range(9):
        nc.tensor.matmul(psq[:], xt[:, c, :], p["wqkv_sb"][:, c, :],
                         start=(c == 0), stop=(c == 8))

    sb = p["p1"].tile([128, 3 * HL * D], F32, tag="sb", name="sb")
    nc.scalar.activation(sb[:], psq[:], AF.Copy)

    # ssr[g] = sum_d qkv[g,d]^2 (6 fused square+reduce)
    ssr = p["p1"].tile([128, 6], F32, tag="ssr", name="ssr")
    sqd = p["p1"].tile([128, D], F32, tag="sqd", name="sqd")
    for g in range(6):
        gs = sb[:, g * D:(g + 1) * D]
        nc.vector.scalar_tensor_tensor(sqd[:], gs, 1.0, gs,
                                       op0=ALU.mult, op1=ALU.mult,
                                       accum_out=ssr[:, g:g + 1])

    # y = rsqrt(ssr) via bit-trick seed + 2 Newton iterations.  Pool only
    # has tensor-tensor mult/add, so precompute hneg=-ssr/2 on DVE and use
    # the 1.5-constant tile: y <- y * (c15 + (y*y)*hneg).
    y = p["p1"].tile([128, 6], F32, tag="y", name="y")
    t1 = p["p1"].tile([128, 6], F32, tag="t1", name="t1")
    hneg = p["p1"].tile([128, 6], F32, tag="hneg", name="hneg")
    nc.vector.tensor_scalar(y[:].bitcast(I32), ssr[:].bitcast(I32),
                            1, None, op0=ALU.logical_shift_right)
    nc.vector.tensor_tensor(y[:].bitcast(I32), p["magic"][:, 0:6],
                            y[:].bitcast(I32), op=ALU.subtract)
    nc.vector.tensor_scalar_mul(hneg[:], ssr[:], -0.5)
    for _ in range(2):
        nc.vector.tensor_mul(t1[:], y[:], y[:])
        nc.vector.tensor_mul(t1[:], t1[:], hneg[:])
        nc.vector.tensor_add(t1[:], t1[:], p["c15"][:, 0:6])
        nc.vector.tensor_mul(y[:], y[:], t1[:])
    nc.vector.tensor_scalar_mul(y[:], y[:], SQ72)

    # rope on q (groups 0:2) and k (groups 2:4): expand per-t cos/sin rows
    # across hl heads (3D broadcast copies), then 6 elementwise ops.
    rc3 = rc.rearrange("p (t s) d -> p t s d", s=2)
    cwx = p["p1"].tile([128, 2, HL, D], F32, tag="cwx", name="cwx")
    swx = p["p1"].tile([128, 2, HL, D], F32, tag="swx", name="swx")
    nc.vector.tensor_copy(
        cwx[:], rc3[:, :, 0, :].unsqueeze(2).to_broadcast([128, 2, HL, D]))
    nc.vector.tensor_copy(
        swx[:], rc3[:, :, 1, :].unsqueeze(2).to_broadcast([128, 2, HL, D]))
    qk5 = sb[:, 0:2 * HL * D].rearrange("p (g a c j) -> p g a c j",
                                        g=2 * HL, a=2, c=2)
    cw5 = cwx[:].rearrange("p t hl (a c j) -> p (t hl) a c j", a=2, c=2)
    sw5 = swx[:].rearrange("p t hl (a c j) -> p (t hl) a c j", a=2, c=2)
    rp = p["p1"].tile([128, 2 * HL, D], F32, tag="rp", name="rp")
    rp5 = rp[:].rearrange("p g (a c j) -> p g a c j", a=2, c=2)
    tmp = p["p1"].tile([128, 2 * HL, 2, 18], F32, tag="rtmp", name="rtmp")
    nc.vector.tensor_mul(rp5[:, :, :, 0, :], qk5[:, :, :, 0, :],
                         cw5[:, :, :, 0, :])
    nc.vector.tensor_mul(tmp[:], qk5[:, :, :, 1, :], sw5[:, :, :, 0, :])
    nc.vector.tensor_sub(rp5[:, :, :, 0, :], rp5[:, :, :, 0, :], tmp[:])
    nc.vector.tensor_mul(rp5[:, :, :, 1, :], qk5[:, :, :, 1, :],
                         cw5[:, :, :, 1, :])
    nc.vector.tensor_mul(tmp[:], qk5[:, :, :, 0, :], sw5[:, :, :, 1, :])
    nc.vector.tensor_add(rp5[:, :, :, 1, :], rp5[:, :, :, 1, :], tmp[:])

    # ss2[hl] = sum_d rope(q_raw)^2
    ss2 = p["p1"].tile([128, HL], F32, tag="ss2", name="ss2")
    for hl in range(HL):
        rs = rp[:, hl, :]
        nc.vector.scalar_tensor_tensor(sqd[:], rs, 1.0, rs,
                                       op0=ALU.mult, op1=ALU.mult,
                                       accum_out=ss2[:, hl:hl + 1])

    # c_q = 8*sqrt(72)*y_q*(ss2*seed(ss2)); c's precision is irrelevant
    # (cancels in softmax), only its range matters.
    y0s = p["p1"].tile([128, HL], F32, tag="y0s", name="y0s")
    m1 = p["p1"].tile([128, HL], F32, tag="m1", name="m1")
    nc.vector.tensor_scalar(y0s[:].bitcast(I32), ss2[:].bitcast(I32),
                            1, None, op0=ALU.logical_shift_right)
    nc.vector.tensor_tensor(y0s[:].bitcast(I32), p["magic"][:, 0:HL],
                            y0s[:].bitcast(I32), op=ALU.subtract)
    nc.vector.tensor_mul(m1[:], ss2[:], y0s[:])
    nc.vector.tensor_mul(m1[:], m1[:], y[:, 0:2])

    qaug = p["p1"].tile([128, HL, D + 1], F32, tag="qaug", name="qaug")
    kaug = p["p1"].tile([128, HL, D + 1], F32, tag="kaug", name="kaug")
    nc.vector.tensor_mul(qaug[:, :, D], m1[:], p["cneg"][:, 0:HL])
    nc.vector.memset(kaug[:, :, D], 1.0)

    ybq = y[:, 0:2].unsqueeze(2).to_broadcast([128, HL, D])
    ybk = y[:, 2:4].unsqueeze(2).to_broadcast([128, HL, D])
    ybv = y[:, 4:6].unsqueeze(2).to_broadcast([128, HL, D])
    nc.vector.tensor_mul(qaug[:, :, 0:D], rp[:, 0:2, :], ybq)
    nc.vector.tensor_mul(kaug[:, :, 0:D], rp[:, 2:4, :], ybk)
    nc.vector.tensor_mul(
        p["vaug"][b][:, kb, :, 0:D],
        sb[:, 2 * HL * D:3 * HL * D].rearrange("p (hl d) -> p hl d", hl=HL),
        ybv)

    # transpose q/k to feature-major; stage q for 4 blocks per DMA
    tr4 = p["trps"].tile([D + 1, 4, 128], F32, tag="tr4", name="tr4")
    for hl in range(HL):
        nc.tensor.transpose(tr4[:, hl, :], qaug[:, hl, :], p["ident"][:])
        nc.tensor.transpose(tr4[:, 2 + hl, :], kaug[:, hl, :], p["ident"][:])
    if kb % 4 == 0:
        ctx["qstg"] = p["p1"].tile([D + 1, HL, 512], F32R, tag="qstg",
                                   name="qstg")
    qs = kb % 4
    nc.vector.tensor_copy(ctx["qstg"][:, :, qs * 128:(qs + 1) * 128],
                          tr4[:, 0:2, :])
    nc.vector.tensor_copy(p["kt"][b][:, :, kb * 128:(kb + 1) * 128],
                          tr4[:, 2:4, :])
    if kb % 4 == 3:
        k4 = kb - 3
        nc.gpsimd.dma_start(
            p["qt_dram"][b][:, :, k4 * 128:(k4 + 4) * 128], ctx["qstg"][:])


def _emit_normalize(nc, ctx, src_ap, b, hl, qc):
    p = ctx
    rec = p["p2n"].tile([1, QC], F32, tag="rec", name="rec")
    nc.vector.reciprocal(rec[:], src_ap[96:97, :])
    bct = p["p2n"].tile([D, QC], F32, tag="bct", name="bct")
    nc.gpsimd.partition_broadcast(bct[:], rec[:])
    onrm = p["p2n"].tile([D, QC], BF16, tag="onrm", name="onrm")
    nc.vector.tensor_mul(onrm[:], src_ap[0:D, :], bct[:])
    e = b * 4 + qc // 2
    h = qc % 2
    nc.gpsimd.dma_start(p["a2a_in"][h][e, hl * D:(hl + 1) * D, :], onrm[:])


def _emit_ph2_iter(nc, ctx, b, hl, qc, defer=False):
    """Full attention for one (batch, head, 512-query chunk)."""
    p = ctx
    qsl = slice(qc * QC, (qc + 1) * QC)
    qt_t = p["p2"].tile([D + 1, QC], F32R, tag="qt", name="qt_t")
    nc.sync.dma_start(qt_t[:], p["qt_dram"][b][:, hl, qsl])

    pso = p["psos"].tile([97, QC], F32, tag="pso", name="pso")
    for kb in range(KBLK):
        ksl = slice(kb * 128, (kb + 1) * 128)
        pss = p["pss"].tile([128, QC], F32, tag="pss", name="pss")
        nc.tensor.matmul(pss[:], p["kt"][b][:, hl, ksl], qt_t[:],
                         start=True, stop=True)
        pt = p["p2"].tile([128, QC], BF16, tag="pt", name="pt")
        nc.scalar.activation(pt[:], pss[:], AF.Exp)
        nc.tensor.matmul(pso[:], p["vaug"][b][:, kb, hl, :], pt[:],
                         start=(kb == 0), stop=(kb == KBLK - 1))

    if defer:
        # stash the raw accumulator; normalize later when DVE is idle
        # (the even-half outputs aren't needed until the first AllToAll)
        st = p["stash"].tile([97, QC], BF16, tag=f"st_{b}_{hl}_{qc}",
                             name=f"st_{b}_{hl}_{qc}")
        nc.vector.tensor_copy(st[:], pso[:])
        p["deferred"].append((st, b, hl, qc))
    else:
        _emit_normalize(nc, ctx, pso[:], b, hl, qc)


def _emit_oproj_half(nc, ctx, tcn):
    p = ctx
    tsl = slice(tcn * QC, (tcn + 1) * QC)
    yt = p["p3"].tile([128, 9, QC], BF16, tag=f"yt{tcn}", name=f"yt{tcn}")
    nc.sync.dma_start(
        yt[:],
        p["a2a_out"][tcn][:].rearrange("j r t -> (j r) t").rearrange(
            "(c p) t -> p c t", p=128))
    for fo in range(9):
        ps3 = p["p3ps"].tile([128, QC], F32, tag="ps3", name="ps3")
        for fi in range(9):
            nc.tensor.matmul(ps3[:],
                             p["wo_sb"][:, fi, fo * 128:(fo + 1) * 128],
                             yt[:, fi, :], start=(fi == 0), stop=(fi == 8))
        ot = p["p3o"].tile([128, QC], F32, tag="ot", name="ot")
        nc.scalar.activation(ot[:], ps3[:], AF.Copy)
        nc.sync.dma_start(p["outT"][:, fo, tsl], ot[:])


def _build_nc():
    nc = bacc.Bacc("TRN2", target_bir_lowering=False, debug=False,
                   num_devices=N_CORES)

    xb = nc.dram_tensor("xb", [128, B * NBLK, 9, 128], F32R,
                        kind="ExternalInput").ap()
    rcb = nc.dram_tensor("rcb", [128, B * NBLK, 4, D], F32,
                         kind="ExternalInput").ap()
    wqkv = nc.dram_tensor("wqkv", [128, 9, 3 * HL * D], F32R,
                          kind="ExternalInput").ap()
    wo = nc.dram_tensor("wo", [128, 9, HID], BF16, kind="ExternalInput").ap()
    outT = nc.dram_tensor("outT", [128, 9, 1024], F32,
                          kind="ExternalOutput").ap()

    with tile.TileContext(nc) as tc:
        with (
            tc.tile_pool(name="persist", bufs=1) as persist,
            tc.tile_pool(name="dram", bufs=1, space="DRAM") as dram,
        ):
            ctx = {"xb": xb, "rcb": rcb, "outT": outT}

            ident = persist.tile([128, 128], F32, tag="ident")
            make_identity(nc, ident)
            ctx["ident"] = ident
            magic = persist.tile([128, 6], I32, tag="magic")
            nc.vector.memset(magic[:], MAGIC)
            ctx["magic"] = magic
            c15 = persist.tile([128, 6], F32, tag="c15")
            nc.vector.memset(c15[:], 1.5)
            ctx["c15"] = c15
            cneg = persist.tile([128, 2], F32, tag="cneg")
            nc.vector.memset(cneg[:], -BETA)
            ctx["cneg"] = cneg

            wqkv_sb = persist.tile([128, 9, 3 * HL * D], F32R, tag="wqkv")
            nc.sync.dma_start(wqkv_sb[:], wqkv)
            ctx["wqkv_sb"] = wqkv_sb

            ctx["kt"] = [persist.tile([D + 1, HL, P], F32R, tag=f"kt{b}",
                                      name=f"kt{b}") for b in range(B)]
            ctx["vaug"] = [persist.tile([128, KBLK, HL, 97], BF16,
                                        tag=f"vaug{b}", name=f"vaug{b}")
                           for b in range(B)]
            for b in range(B):
                nc.vector.memset(ctx["vaug"][b][:], 0.0)
                nc.vector.memset(ctx["vaug"][b][:, :, :, 96], 1.0)

            ctx["qt_dram"] = [dram.tile([D + 1, HL, P], F32R,
                                        name=f"qtd{b}") for b in range(B)]
            ctx["a2a_in"] = [dram.tile([N_CORES, HL * D, QC], BF16,
                                       name=f"a2ai{h}") for h in range(2)]
            ctx["a2a_out"] = [dram.tile([N_CORES, HL * D, QC], BF16,
                                        name=f"a2ao{h}") for h in range(2)]

            evens = [0, 2, 4, 6]
            odds = [1, 3, 5, 7]

            ctx["deferred"] = []
            with (
                tc.tile_pool(name="stash", bufs=1) as stash,
                tc.tile_pool(name="p2", bufs=4) as p2,
                tc.tile_pool(name="p2n", bufs=2) as p2n,
                tc.tile_pool(name="pss", bufs=4, space="PSUM") as pss,
                tc.tile_pool(name="psos", bufs=2, space="PSUM") as psos,
            ):
                ctx.update(p2=p2, p2n=p2n, pss=pss, psos=psos, stash=stash)

                # ---- phase 1 (b=0), then phase2(b=0 evens) x phase1(b=1)
                with (
                    tc.tile_pool(name="p1", bufs=3) as p1,
                    tc.tile_pool(name="p1ps", bufs=1, space="PSUM") as p1ps,
                    tc.tile_pool(name="trps", bufs=1, space="PSUM") as trps,
                ):
                    ctx.update(p1=p1, p1ps=p1ps, trps=trps)
                    for kbp in range(NBLK // 2):
                        _emit_ph1_pair(nc, ctx, 0, kbp)
                    for j, qc in enumerate(evens):
                        for hl in range(HL):
                            _emit_ph2_iter(nc, ctx, 0, hl, qc)
                            _emit_ph1_pair(nc, ctx, 1, 2 * (2 * j + hl))
                            _emit_ph1_pair(nc, ctx, 1, 2 * (2 * j + hl) + 1)

                for qc in evens:
                    for hl in range(HL):
                        _emit_ph2_iter(nc, ctx, 1, hl, qc)

                nc.gpsimd.collective_compute(
                    "AllToAll", mybir.AluOpType.bypass,
                    ins=[ctx["a2a_in"][0][:]],
                    outs=[ctx["a2a_out"][0][:]],
                    replica_groups=[list(range(N_CORES))],
                )

                with (
                    tc.tile_pool(name="p3", bufs=1) as p3,
                    tc.tile_pool(name="p3o", bufs=2) as p3o,
                    tc.tile_pool(name="p3ps", bufs=2, space="PSUM") as p3ps,
                ):
                    ctx.update(p3=p3, p3o=p3o, p3ps=p3ps)
                    wo_sb = p3.tile([128, 9, HID], BF16, tag="wo",
                                    name="wo_sb")
                    nc.sync.dma_start(wo_sb[:], wo)
                    ctx["wo_sb"] = wo_sb

                    for b in range(B):
                        for qc in odds:
                            for hl in range(HL):
                                _emit_ph2_iter(nc, ctx, b, hl, qc)

                    _emit_oproj_half(nc, ctx, 0)

                    nc.gpsimd.collective_compute(
                        "AllToAll", mybir.AluOpType.bypass,
                        ins=[ctx["a2a_in"][1][:]],
                        outs=[ctx["a2a_out"][1][:]],
                        replica_groups=[list(range(N_CORES))],
                    )

                    _emit_oproj_half(nc, ctx, 1)

    nc.compile()
    return nc


def _prep_inputs(inputs):
    hs = np.asarray(inputs["hidden_states"], dtype=np.float32)
    cos = np.asarray(inputs["cos"], dtype=np.float32)
    sin = np.asarray(inputs["sin"], dtype=np.float32)
    Wq = np.asarray(inputs["Wq"], dtype=np.float32)
    Wk = np.asarray(inputs["Wk"], dtype=np.float32)
    Wv = np.asarray(inputs["Wv"], dtype=np.float32)
    Wo = np.asarray(inputs["Wo"], dtype=np.float32)
    qw = np.asarray(inputs["q_norm_w"], dtype=np.float32)
    kw = np.asarray(inputs["k_norm_w"], dtype=np.float32)

    # x blocked: xb[p, blk, c, ti] = x[t=blk*128+ti, h=c*128+p]
    xb = np.ascontiguousarray(
        hs.reshape(TB // 128, 128, 9, 128).transpose(3, 0, 2, 1))

    # rope coefficients per token: rows [cwq, swq, cwk, swk]
    partner = np.empty(D, np.int64)
    for a in range(2):
        base = a * 36
        partner[base:base + 18] = np.arange(base + 18, base + 36)
        partner[base + 18:base + 36] = np.arange(base, base + 18)
    cs = cos.reshape(TB, D)
    sn = sin.reshape(TB, D)
    ropec = np.stack([cs * qw[None, :], sn * qw[partner][None, :],
                      cs * kw[None, :], sn * kw[partner][None, :]],
                     axis=1).astype(np.float32)
    rcb = np.ascontiguousarray(
        ropec.reshape(TB // 128, 128, 4, D).transpose(1, 0, 2, 3))

    wob = np.ascontiguousarray(
        Wo.reshape(9, 128, HID).transpose(1, 0, 2)).astype(
            ml_dtypes.bfloat16)

    in_maps = []
    for c in range(N_CORES):
        colsl = slice(c * HL * D, (c + 1) * HL * D)
        wqkv = np.concatenate([Wq[:, colsl], Wk[:, colsl], Wv[:, colsl]],
                              axis=1)
        wqkv = np.ascontiguousarray(
            wqkv.reshape(9, 128, 3 * HL * D).transpose(1, 0, 2))
        in_maps.append({
            "xb": xb,
            "rcb": rcb,
            "wqkv": wqkv,
            "wo": wob,
        })
    return in_maps


def kernel(**inputs):
    global _CACHED_NC
    if _CACHED_NC is None:
        _CACHED_NC = _build_nc()
    nc = _CACHED_NC
    in_maps = _prep_inputs(inputs)
    trace = bool(int(os.environ.get("KERNEL_TRACE", "0")))
    res = run_bass_kernel_spmd(nc, in_maps, core_ids=list(range(N_CORES)),
                               trace=trace)
    kernel.last_results = res
    out = np.empty((B, P, HID), dtype=np.float32)
    for c in range(N_CORES):
        b, qtr = c // 4, c % 4
        r = res.results[c]["outT"]  # [128, 9, 1024]
        out[b, qtr * 1024:(qtr + 1) * 1024, :] = \
            np.asarray(r).transpose(2, 1, 0).reshape(1024, HID)
    return out


# revision 39
# speedup vs baseline: 1.0547x; 1.0107x over previous
"""Trainium2 Bass kernel for NeuronGemma4VisionAttention.

Problem: B=2, P=4096, HID=1152, 16 heads x 72 dim, fp32.
  q,k,v = x@Wq, x@Wk, x@Wv  -> per-head RMSNorm (q,k learned scale, v none)
  -> 2-part RoPE on q,k -> softmax(q k^T) v -> concat heads @ Wo

Sharding (8 cores, one chip):
  Head-parallel: core c owns heads (2c, 2c+1) for BOTH batches.
  Each core: QKV projection (its 144 columns of each W), per-head norm+rope,
  full non-causal attention for its 2 heads x 2 batches; two half-token
  AllToAlls exchange token-eighths (second one overlapped with o_proj of the
  first half) so core c ends with the full 1152-dim attention output for
  tokens [1024*(c%4) ...) of batch c//4, on which it runs the o_proj.

Key implementation notes (v3):
  - Scalar engine runs ONLY the softmax Exp (plus a few copies): RMSNorm
    rsqrt is a bit-trick+Newton on DVE/Pool, the softmax stabilizer c_q uses
    a seed-only sqrt (c cancels in softmax; only its range matters --
    validated max(rowmax-c)=32<80).
  - Exp in [128,512] single-PSUM-bank tiles (bank-crossing reads are slower).
  - P (exp scores), V, A2A payload, o_proj in bf16 (e2e 3.1e-3); q/k/scores
    stay f32r (bf16 there would be 1.2-1.5e-2, too close to the 2e-2 gate).
  - Softmax denominators via the accurate DVE reciprocal macro: they span
    down to 1e-27 (c overshoots rowmax by up to ~60), where
    reciprocal_approx_fast returns garbage.
  - DMA: host pre-blocks x/ropec so descriptors are >=2.3KB; x loads in
    2-block groups on the SP queue, ropec on the Act queue, qt
    staging written in 4-block groups and all-to-all payload on the Pool
    (SWDGE) queue -- one HWDGE queue was the phase-1 bottleneck before.
  - Phase1(b=1) emission interleaved with phase2(b=0); pso double-buffered
    so the normalize/reciprocal chain is off the PE critical path.
"""
import os
import sys

sys.path.insert(0, "/opt/trn_rl_repo")

import numpy as np
import ml_dtypes

import concourse.bass as bass  # noqa: F401
import concourse.tile as tile
from concourse import bacc, mybir
from concourse.bass_utils import run_bass_kernel_spmd
from concourse.masks import make_identity

F32 = mybir.dt.float32
F32R = mybir.dt.float32r
BF16 = mybir.dt.bfloat16
I32 = mybir.dt.int32
AF = mybir.ActivationFunctionType
ALU = mybir.AluOpType

N_CORES = 8
B, P, HID = 2, 4096, 1152
NH, D = 16, 72
HL = 2                 # heads per core
TB = B * P
NBLK = P // 128        # 32 token blocks per batch
QC = 512               # query chunk
KBLK = P // 128        # 32 key blocks per batch
SQ72 = 8.48528137423857   # sqrt(72)
BETA = 8.0
MAGIC = 0x5F3759DF

_CACHED_NC = None


def _emit_ph1_pair(nc, ctx, b, kbp):
    """Two 128-token blocks: loads grouped, compute per block."""
    p = ctx
    gpair = b * NBLK + 2 * kbp
    xt = p["p1"].tile([128, 2, 9, 128], F32R, tag="xt", name="xt")
    nc.sync.dma_start(xt[:], p["xb"][:, gpair:gpair + 2])
    rc2 = p["p1"].tile([128, 2, 4, D], F32, tag="rc", name="rc2")
    nc.scalar.dma_start(rc2[:], p["rcb"][:, gpair:gpair + 2])

    for half in range(2):
        kb = 2 * kbp + half
        _emit_ph1_block(nc, ctx, b, kb, xt[:, half], rc2[:, half])


def _emit_ph1_block(nc, ctx, b, kb, xt, rc):
    p = ctx

    psq = p["p1ps"].tile([128, 3 * HL * D], F32, tag="psq", name="psq")
    for c in range(9):
        nc.tensor.matmul(psq[:], xt[:, c, :], p["wqkv_sb"][:, c, :],
                         start=(c == 0), stop=(c == 8))

    sb = p["p1"].tile([128, 3 * HL * D], F32, tag="sb", name="sb")
    nc.scalar.activation(sb[:], psq[:], AF.Copy)

    # ssr[g] = sum_d qkv[g,d]^2 (6 fused square+reduce)
    ssr = p["p1"].tile([128, 6], F32, tag="ssr", name="ssr")
    sqd = p["p1"].tile([128, D], F32, tag="sqd", name="sqd")
    for g in range(6):
        gs = sb[:, g * D:(g + 1) * D]
        nc.vector.scalar_tensor_tensor(sqd[:], gs, 1.0, gs,
                                       op0=ALU.mult, op1=ALU.mult,
                                       accum_out=ssr[:, g:g + 1])

    # y = rsqrt(ssr) via bit-trick seed + 2 Newton iterations.  Pool only
    # has tensor-tensor mult/add, so precompute hneg=-ssr/2 on DVE and use
    # the 1.5-constant tile: y <- y * (c15 + (y*y)*hneg).
    y = p["p1"].tile([128, 6], F32, tag="y", name="y")
    t1 = p["p1"].tile([128, 6], F32, tag="t1", name="t1")
    hneg = p["p1"].tile([128, 6], F32, tag="hneg", name="hneg")
    nc.vector.tensor_scalar(y[:].bitcast(I32), ssr[:].bitcast(I32),
                            1, None, op0=ALU.logical_shift_right)
    nc.vector.tensor_tensor(y[:].bitcast(I32), p["magic"][:, 0:6],
                            y[:].bitcast(I32), op=ALU.subtract)
    nc.vector.tensor_scalar_mul(hneg[:], ssr[:], -0.5)
    for _ in range(2):
        nc.vector.tensor_mul(t1[:], y[:], y[:])
        nc.vector.tensor_mul(t1[:], t1[:], hneg[:])
        nc.vector.tensor_add(t1[:], t1[:], p["c15"][:, 0:6])
        nc.vector.tensor_mul(y[:], y[:], t1[:])
    nc.vector.tensor_scalar_mul(y[:], y[:], SQ72)

    # rope on q (groups 0:2) and k (groups 2:4): expand per-t cos/sin rows
    # across hl heads (3D broadcast copies), then 6 elementwise ops.
    rc3 = rc.rearrange("p (t s) d -> p t s d", s=2)
    cwx = p["p1"].tile([128, 2, HL, D], F32, tag="cwx", name="cwx")
    swx = p["p1"].tile([128, 2, HL, D], F32, tag="swx", name="swx")
    nc.vector.tensor_copy(
        cwx[:], rc3[:, :, 0, :].unsqueeze(2).to_broadcast([128, 2, HL, D]))
    nc.vector.tensor_copy(
        swx[:], rc3[:, :, 1, :].unsqueeze(2).to_broadcast([128, 2, HL, D]))
    qk5 = sb[:, 0:2 * HL * D].rearrange("p (g a c j) -> p g a c j",
                                        g=2 * HL, a=2, c=2)
    cw5 = cwx[:].rearrange("p t hl (a c j) -> p (t hl) a c j", a=2, c=2)
    sw5 = swx[:].rearrange("p t hl (a c j) -> p (t hl) a c j", a=2, c=2)
    rp = p["p1"].tile([128, 2 * HL, D], F32, tag="rp", name="rp")
    rp5 = rp[:].rearrange("p g (a c j) -> p g a c j", a=2, c=2)
    tmp = p["p1"].tile([128, 2 * HL, 2, 18], F32, tag="rtmp", name="rtmp")
    nc.vector.tensor_mul(rp5[:, :, :, 0, :], qk5[:, :, :, 0, :],
                         cw5[:, :, :, 0, :])
    nc.vector.tensor_mul(tmp[:], qk5[:, :, :, 1, :], sw5[:, :, :, 0, :])
    nc.vector.tensor_sub(rp5[:, :, :, 0, :], rp5[:, :, :, 0, :], tmp[:])
    nc.vector.tensor_mul(rp5[:, :, :, 1, :], qk5[:, :, :, 1, :],
                         cw5[:, :, :, 1, :])
    nc.vector.tensor_mul(tmp[:], qk5[:, :, :, 0, :], sw5[:, :, :, 1, :])
    nc.vector.tensor_add(rp5[:, :, :, 1, :], rp5[:, :, :, 1, :], tmp[:])

    # ss2[hl] = sum_d rope(q_raw)^2
    ss2 = p["p1"].tile([128, HL], F32, tag="ss2", name="ss2")
    for hl in range(HL):
        rs = rp[:, hl, :]
        nc.vector.scalar_tensor_tensor(sqd[:], rs, 1.0, rs,
                                       op0=ALU.mult, op1=ALU.mult,
                                       accum_out=ss2[:, hl:hl + 1])

    # c_q = 8*sqrt(72)*y_q*(ss2*seed(ss2)); c's precision is irrelevant
    # (cancels in softmax), only its range matters.
    y0s = p["p1"].tile([128, HL], F32, tag="y0s", name="y0s")
    m1 = p["p1"].tile([128, HL], F32, tag="m1", name="m1")
    nc.vector.tensor_scalar(y0s[:].bitcast(I32), ss2[:].bitcast(I32),
                            1, None, op0=ALU.logical_shift_right)
    nc.vector.tensor_tensor(y0s[:].bitcast(I32), p["magic"][:, 0:HL],
                            y0s[:].bitcast(I32), op=ALU.subtract)
    nc.vector.tensor_mul(m1[:], ss2[:], y0s[:])
    nc.vector.tensor_mul(m1[:], m1[:], y[:, 0:2])

    qaug = p["p1"].tile([128, HL, D + 1], F32, tag="qaug", name="qaug")
    kaug = p["p1"].tile([128, HL, D + 1], F32, tag="kaug", name="kaug")
    nc.vector.tensor_mul(qaug[:, :, D], m1[:], p["cneg"][:, 0:HL])
    nc.vector.memset(kaug[:, :, D], 1.0)

    ybq = y[:, 0:2].unsqueeze(2).to_broadcast([128, HL, D])
    ybk = y[:, 2:4].unsqueeze(2).to_broadcast([128, HL, D])
    ybv = y[:, 4:6].unsqueeze(2).to_broadcast([128, HL, D])
    nc.vector.tensor_mul(qaug[:, :, 0:D], rp[:, 0:2, :], ybq)
    nc.vector.tensor_mul(kaug[:, :, 0:D], rp[:, 2:4, :], ybk)
    nc.vector.tensor_mul(
        p["vaug"][b][:, kb, :, 0:D],
        sb[:, 2 * HL * D:3 * HL * D].rearrange("p (hl d) -> p hl d", hl=HL),
        ybv)

    # transpose q/k to feature-major; stage q for 4 blocks per DMA
    tr4 = p["trps"].tile([D + 1, 4, 128], F32, tag="tr4", name="tr4")
    for hl in range(HL):
        nc.tensor.transpose(tr4[:, hl, :], qaug[:, hl, :], p["ident"][:])
        nc.tensor.transpose(tr4[:, 2 + hl, :], kaug[:, hl, :], p["ident"][:])
    if kb % 4 == 0:
        ctx["qstg"] = p["p1"].tile([D + 1, HL, 512], F32R, tag="qstg",
                                   name="qstg")
    qs = kb % 4
    nc.vector.tensor_copy(ctx["qstg"][:, :, qs * 128:(qs + 1) * 128],
                          tr4[:, 0:2, :])
    nc.vector.tensor_copy(p["kt"][b][:, :, kb * 128:(kb + 1) * 128],
                          tr4[:, 2:4, :])
    if kb % 4 == 3:
        k4 = kb - 3
        nc.gpsimd.dma_start(
            p["qt_dram"][b][:, :, k4 * 128:(k4 + 4) * 128], ctx["qstg"][:])


def _emit_normalize(nc, ctx, src_ap, b, hl, qc):
    p = ctx
    rec = p["p2n"].tile([1, QC], F32, tag="rec", name="rec")
    nc.vector.reciprocal(rec[:], src_ap[96:97, :])
    bct = p["p2n"].tile([D, QC], F32, tag="bct", name="bct")
    nc.gpsimd.partition_broadcast(bct[:], rec[:])
    onrm = p["p2n"].tile([D, QC], BF16, tag="onrm", name="onrm")
    nc.vector.tensor_mul(onrm[:], src_ap[0:D, :], bct[:])
    e = b * 4 + qc // 2
    h = qc % 2
    nc.gpsimd.dma_start(p["a2a_in"][h][e, hl * D:(hl + 1) * D, :], onrm[:])


def _emit_ph2_iter(nc, ctx, b, hl, qc, defer=False):
    """Full attention for one (batch, head, 512-query chunk)."""
    p = ctx
    qsl = slice(qc * QC, (qc + 1) * QC)
    qt_t = p["p2"].tile([D + 1, QC], F32R, tag="qt", name="qt_t")
    nc.sync.dma_start(qt_t[:], p["qt_dram"][b][:, hl, qsl])

    pso = p["psos"].tile([97, QC], F32, tag="pso", name="pso")
    for kb in range(KBLK):
        ksl = slice(kb * 128, (kb + 1) * 128)
        pss = p["pss"].tile([128, QC], F32, tag="pss", name="pss")
        nc.tensor.matmul(pss[:], p["kt"][b][:, hl, ksl], qt_t[:],
                         start=True, stop=True)
        pt = p["p2"].tile([128, QC], BF16, tag="pt", name="pt")
        nc.scalar.activation(pt[:], pss[:], AF.Exp)
        nc.tensor.matmul(pso[:], p["vaug"][b][:, kb, hl, :], pt[:],
                         start=(kb == 0), stop=(kb == KBLK - 1))

    if defer:
        # stash the raw accumulator; normalize later when DVE is idle
        # (the even-half outputs aren't needed until the first AllToAll)
        st = p["stash"].tile([97, QC], BF16, tag=f"st_{b}_{hl}_{qc}",
                             name=f"st_{b}_{hl}_{qc}")
        nc.vector.tensor_copy(st[:], pso[:])
        p["deferred"].append((st, b, hl, qc))
    else:
        _emit_normalize(nc, ctx, pso[:], b, hl, qc)


def _emit_oproj_half(nc, ctx, tcn):
    p = ctx
    tsl = slice(tcn * QC, (tcn + 1) * QC)
    yt = p["p3"].tile([128, 9, QC], BF16, tag=f"yt{tcn}", name=f"yt{tcn}")
    ysrc = p["a2a_out"][tcn][:].rearrange("j r t -> (j r) t").rearrange(
        "(c p) t -> p c t", p=128)
    for fi in range(9):
        nc.sync.dma_start(yt[:, fi, :], ysrc[:, fi, :])
    for fo in range(9):
        ps3 = p["p3ps"].tile([128, QC], F32, tag="ps3", name="ps3")
        for fi in range(9):
            nc.tensor.matmul(ps3[:],
                             p["wo_sb"][:, fi, fo * 128:(fo + 1) * 128],
                             yt[:, fi, :], start=(fi == 0), stop=(fi == 8))
        ot = p["p3o"].tile([128, QC], F32, tag="ot", name="ot")
        nc.scalar.activation(ot[:], ps3[:], AF.Copy)
        nc.sync.dma_start(p["outT"][:, fo, tsl], ot[:])


def _build_nc():
    nc = bacc.Bacc("TRN2", target_bir_lowering=False, debug=False,
                   num_devices=N_CORES)

    xb = nc.dram_tensor("xb", [128, B * NBLK, 9, 128], F32R,
                        kind="ExternalInput").ap()
    rcb = nc.dram_tensor("rcb", [128, B * NBLK, 4, D], F32,
                         kind="ExternalInput").ap()
    wqkv = nc.dram_tensor("wqkv", [128, 9, 3 * HL * D], F32R,
                          kind="ExternalInput").ap()
    wo = nc.dram_tensor("wo", [128, 9, HID], BF16, kind="ExternalInput").ap()
    outT = nc.dram_tensor("outT", [128, 9, 1024], F32,
                          kind="ExternalOutput").ap()

    with tile.TileContext(nc) as tc:
        with (
            tc.tile_pool(name="persist", bufs=1) as persist,
            tc.tile_pool(name="dram", bufs=1, space="DRAM") as dram,
        ):
            ctx = {"xb": xb, "rcb": rcb, "outT": outT}

            ident = persist.tile([128, 128], F32, tag="ident")
            make_identity(nc, ident)
            ctx["ident"] = ident
            magic = persist.tile([128, 6], I32, tag="magic")
            nc.vector.memset(magic[:], MAGIC)
            ctx["magic"] = magic
            c15 = persist.tile([128, 6], F32, tag="c15")
            nc.vector.memset(c15[:], 1.5)
            ctx["c15"] = c15
            cneg = persist.tile([128, 2], F32, tag="cneg")
            nc.vector.memset(cneg[:], -BETA)
            ctx["cneg"] = cneg

            wqkv_sb = persist.tile([128, 9, 3 * HL * D], F32R, tag="wqkv")
            nc.sync.dma_start(wqkv_sb[:], wqkv)
            ctx["wqkv_sb"] = wqkv_sb

            ctx["kt"] = [persist.tile([D + 1, HL, P], F32R, tag=f"kt{b}",
                                      name=f"kt{b}") for b in range(B)]
            ctx["vaug"] = [persist.tile([128, KBLK, HL, 97], BF16,
                                        tag=f"vaug{b}", name=f"vaug{b}")
                           for b in range(B)]
            for b in range(B):
                nc.vector.memset(ctx["vaug"][b][:], 0.0)
                nc.vector.memset(ctx["vaug"][b][:, :, :, 96], 1.0)

            ctx["qt_dram"] = [dram.tile([D + 1, HL, P], F32R,
                                        name=f"qtd{b}") for b in range(B)]
            ctx["a2a_in"] = [dram.tile([N_CORES, HL * D, QC], BF16,
                                       name=f"a2ai{h}") for h in range(2)]
            ctx["a2a_out"] = [dram.tile([N_CORES, HL * D, QC], BF16,
                                        name=f"a2ao{h}") for h in range(2)]

            evens = [0, 2, 4, 6]
            odds = [1, 3, 5, 7]

            ctx["deferred"] = []
            with (
                tc.tile_pool(name="stash", bufs=1) as stash,
                tc.tile_pool(name="p2", bufs=4) as p2,
                tc.tile_pool(name="p2n", bufs=2) as p2n,
                tc.tile_pool(name="pss", bufs=4, space="PSUM") as pss,
                tc.tile_pool(name="psos", bufs=2, space="PSUM") as psos,
            ):
                ctx.update(p2=p2, p2n=p2n, pss=pss, psos=psos, stash=stash)

                # ---- phase 1 (b=0), then phase2(b=0 evens) x phase1(b=1)
                with (
                    tc.tile_pool(name="p1", bufs=3) as p1,
                    tc.tile_pool(name="p1ps", bufs=1, space="PSUM") as p1ps,
                    tc.tile_pool(name="trps", bufs=1, space="PSUM") as trps,
                ):
                    ctx.update(p1=p1, p1ps=p1ps, trps=trps)
                    for kbp in range(NBLK // 2):
                        _emit_ph1_pair(nc, ctx, 0, kbp)
                    for j, qc in enumerate(evens):
                        for hl in range(HL):
                            _emit_ph2_iter(nc, ctx, 0, hl, qc)
                            _emit_ph1_pair(nc, ctx, 1, 2 * (2 * j + hl))
                            _emit_ph1_pair(nc, ctx, 1, 2 * (2 * j + hl) + 1)

                for qc in evens:
                    for hl in range(HL):
                        _emit_ph2_iter(nc, ctx, 1, hl, qc)

                nc.gpsimd.collective_compute(
                    "AllToAll", mybir.AluOpType.bypass,
                    ins=[ctx["a2a_in"][0][:]],
                    outs=[ctx["a2a_out"][0][:]],
                    replica_groups=[list(range(N_CORES))],
                )

                with (
                    tc.tile_pool(name="p3", bufs=1) as p3,
                    tc.tile_pool(name="p3o", bufs=2) as p3o,
                    tc.tile_pool(name="p3ps", bufs=2, space="PSUM") as p3ps,
                ):
                    ctx.update(p3=p3, p3o=p3o, p3ps=p3ps)
                    wo_sb = p3.tile([128, 9, HID], BF16, tag="wo",
                                    name="wo_sb")
                    nc.sync.dma_start(wo_sb[:], wo)
                    ctx["wo_sb"] = wo_sb

                    for b in range(B):
                        for qc in odds:
                            for hl in range(HL):
                                _emit_ph2_iter(nc, ctx, b, hl, qc)

                    _emit_oproj_half(nc, ctx, 0)

                    nc.gpsimd.collective_compute(
                        "AllToAll", mybir.AluOpType.bypass,
                        ins=[ctx["a2a_in"][1][:]],
                        outs=[ctx["a2a_out"][1][:]],
                        replica_groups=[list(range(N_CORES))],
                    )

                    _emit_oproj_half(nc, ctx, 1)

    nc.compile()
    return nc


def _prep_inputs(inputs):
    hs = np.asarray(inputs["hidden_states"], dtype=np.float32)
    cos = np.asarray(inputs["cos"], dtype=np.float32)
    sin = np.asarray(inputs["sin"], dtype=np.float32)
    Wq = np.asarray(inputs["Wq"], dtype=np.float32)
    Wk = np.asarray(inputs["Wk"], dtype=np.float32)
    Wv = np.asarray(inputs["Wv"], dtype=np.float32)
    Wo = np.asarray(inputs["Wo"], dtype=np.float32)
    qw = np.asarray(inputs["q_norm_w"], dtype=np.float32)
    kw = np.asarray(inputs["k_norm_w"], dtype=np.float32)

    # x blocked: xb[p, blk, c, ti] = x[t=blk*128+ti, h=c*128+p]
    xb = np.ascontiguousarray(
        hs.reshape(TB // 128, 128, 9, 128).transpose(3, 0, 2, 1))

    # rope coefficients per token: rows [cwq, swq, cwk, swk]
    partner = np.empty(D, np.int64)
    for a in range(2):
        base = a * 36
        partner[base:base + 18] = np.arange(base + 18, base + 36)
        partner[base + 18:base + 36] = np.arange(base, base + 18)
    cs = cos.reshape(TB, D)
    sn = sin.reshape(TB, D)
    ropec = np.stack([cs * qw[None, :], sn * qw[partner][None, :],
                      cs * kw[None, :], sn * kw[partner][None, :]],
                     axis=1).astype(np.float32)
    rcb = np.ascontiguousarray(
        ropec.reshape(TB // 128, 128, 4, D).transpose(1, 0, 2, 3))

    wob = np.ascontiguousarray(
        Wo.reshape(9, 128, HID).transpose(1, 0, 2)).astype(
            ml_dtypes.bfloat16)

    in_maps = []
    for c in range(N_CORES):
        colsl = slice(c * HL * D, (c + 1) * HL * D)
        wqkv = np.concatenate([Wq[:, colsl], Wk[:, colsl], Wv[:, colsl]],
                              axis=1)
        wqkv = np.ascontiguousarray(
            wqkv.reshape(9, 128, 3 * HL * D).transpose(1, 0, 2))
        in_maps.append({
            "xb": xb,
            "rcb": rcb,
            "wqkv": wqkv,
            "wo": wob,
        })
    return in_maps


def kernel(**inputs):
    global _CACHED_NC
    if _CACHED_NC is None:
        _CACHED_NC = _build_nc()
    nc = _CACHED_NC
    in_maps = _prep_inputs(inputs)
    trace = bool(int(os.environ.get("KERNEL_TRACE", "0")))
    res = run_bass_kernel_spmd(nc, in_maps, core_ids=list(range(N_CORES)),
                               trace=trace)
    kernel.last_results = res
    out = np.empty((B, P, HID), dtype=np.float32)
    for c in range(N_CORES):
        b, qtr = c // 4, c % 4
        r = res.results[c]["outT"]  # [128, 9, 1024]
        out[b, qtr * 1024:(qtr + 1) * 1024, :] = \
            np.asarray(r).transpose(2, 1, 0).reshape(1024, HID)
    return out


# revision 40
# speedup vs baseline: 1.0771x; 1.0213x over previous
"""Trainium2 Bass kernel for NeuronGemma4VisionAttention.

Problem: B=2, P=4096, HID=1152, 16 heads x 72 dim, fp32.
  q,k,v = x@Wq, x@Wk, x@Wv  -> per-head RMSNorm (q,k learned scale, v none)
  -> 2-part RoPE on q,k -> softmax(q k^T) v -> concat heads @ Wo

Sharding (8 cores, one chip):
  Head-parallel: core c owns heads (2c, 2c+1) for BOTH batches.
  Each core: QKV projection (its 144 columns of each W), per-head norm+rope,
  full non-causal attention for its 2 heads x 2 batches; two half-token
  AllToAlls exchange token-eighths (second one overlapped with o_proj of the
  first half) so core c ends with the full 1152-dim attention output for
  tokens [1024*(c%4) ...) of batch c//4, on which it runs the o_proj.

Key implementation notes (v3):
  - Scalar engine runs ONLY the softmax Exp (plus a few copies): RMSNorm
    rsqrt is a bit-trick+Newton on DVE/Pool, the softmax stabilizer c_q uses
    a seed-only sqrt (c cancels in softmax; only its range matters --
    validated max(rowmax-c)=32<80).
  - Exp in [128,512] single-PSUM-bank tiles (bank-crossing reads are slower).
  - P (exp scores), V, A2A payload, o_proj in bf16 (e2e 3.1e-3); q/k/scores
    stay f32r (bf16 there would be 1.2-1.5e-2, too close to the 2e-2 gate).
  - Softmax denominators via the accurate DVE reciprocal macro: they span
    down to 1e-27 (c overshoots rowmax by up to ~60), where
    reciprocal_approx_fast returns garbage.
  - DMA: host pre-blocks x/ropec so descriptors are >=2.3KB; x loads in
    2-block groups on the SP queue, ropec on the Act queue, qt
    staging written in 4-block groups and all-to-all payload on the Pool
    (SWDGE) queue -- one HWDGE queue was the phase-1 bottleneck before.
  - Phase1(b=1) emission interleaved with phase2(b=0); pso double-buffered
    so the normalize/reciprocal chain is off the PE critical path.
"""
import os
import sys

sys.path.insert(0, "/opt/trn_rl_repo")

import numpy as np
import ml_dtypes

import concourse.bass as bass  # noqa: F401
import concourse.tile as tile
from concourse import bacc, mybir
from concourse.bass_utils import run_bass_kernel_spmd
from concourse.masks import make_identity

F32 = mybir.dt.float32
F32R = mybir.dt.float32r
BF16 = mybir.dt.bfloat16
I32 = mybir.dt.int32
AF = mybir.ActivationFunctionType
ALU = mybir.AluOpType

N_CORES = 8
B, P, HID = 2, 4096, 1152
NH, D = 16, 72
HL = 2                 # heads per core
TB = B * P
NBLK = P // 128        # 32 token blocks per batch
QC = 512               # query chunk
KBLK = P // 128        # 32 key blocks per batch
SQ72 = 8.48528137423857   # sqrt(72)
BETA = 8.0
MAGIC = 0x5F3759DF

_CACHED_NC = None


def _emit_ph1_pair(nc, ctx, b, kbp):
    """Two 128-token blocks: loads grouped, compute per block."""
    p = ctx
    gpair = b * NBLK + 2 * kbp
    xt = p["p1"].tile([128, 2, 9, 128], F32R, tag="xt", name="xt")
    nc.sync.dma_start(xt[:], p["xb"][:, gpair:gpair + 2])
    rc2 = p["p1"].tile([128, 2, 4, D], F32, tag="rc", name="rc2")
    nc.scalar.dma_start(rc2[:], p["rcb"][:, gpair:gpair + 2])

    for half in range(2):
        kb = 2 * kbp + half
        _emit_ph1_block(nc, ctx, b, kb, xt[:, half], rc2[:, half])


def _emit_ph1_block(nc, ctx, b, kb, xt, rc):
    p = ctx

    psq = p["p1ps"].tile([128, 3 * HL * D], F32, tag="psq", name="psq")
    for c in range(9):
        nc.tensor.matmul(psq[:], xt[:, c, :], p["wqkv_sb"][:, c, :],
                         start=(c == 0), stop=(c == 8))

    sb = p["p1"].tile([128, 3 * HL * D], F32, tag="sb", name="sb")
    nc.scalar.activation(sb[:], psq[:], AF.Copy)

    # ssr[g] = sum_d qkv[g,d]^2 (6 fused square+reduce)
    ssr = p["p1"].tile([128, 6], F32, tag="ssr", name="ssr")
    sqd = p["p1"].tile([128, D], F32, tag="sqd", name="sqd")
    for g in range(6):
        gs = sb[:, g * D:(g + 1) * D]
        nc.vector.scalar_tensor_tensor(sqd[:], gs, 1.0, gs,
                                       op0=ALU.mult, op1=ALU.mult,
                                       accum_out=ssr[:, g:g + 1])

    # y = rsqrt(ssr) via bit-trick seed + 2 Newton iterations.  Pool only
    # has tensor-tensor mult/add, so precompute hneg=-ssr/2 on DVE and use
    # the 1.5-constant tile: y <- y * (c15 + (y*y)*hneg).
    y = p["p1"].tile([128, 6], F32, tag="y", name="y")
    t1 = p["p1"].tile([128, 6], F32, tag="t1", name="t1")
    hneg = p["p1"].tile([128, 6], F32, tag="hneg", name="hneg")
    nc.vector.tensor_scalar(y[:].bitcast(I32), ssr[:].bitcast(I32),
                            1, None, op0=ALU.logical_shift_right)
    nc.vector.tensor_tensor(y[:].bitcast(I32), p["magic"][:, 0:6],
                            y[:].bitcast(I32), op=ALU.subtract)
    nc.vector.tensor_scalar_mul(hneg[:], ssr[:], -0.5)
    for _ in range(2):
        nc.vector.tensor_mul(t1[:], y[:], y[:])
        nc.vector.tensor_mul(t1[:], t1[:], hneg[:])
        nc.vector.tensor_add(t1[:], t1[:], p["c15"][:, 0:6])
        nc.vector.tensor_mul(y[:], y[:], t1[:])
    nc.vector.tensor_scalar_mul(y[:], y[:], SQ72)

    # rope on q (groups 0:2) and k (groups 2:4): expand per-t cos/sin rows
    # across hl heads (3D broadcast copies), then 6 elementwise ops.
    rc3 = rc.rearrange("p (t s) d -> p t s d", s=2)
    cwx = p["p1"].tile([128, 2, HL, D], F32, tag="cwx", name="cwx")
    swx = p["p1"].tile([128, 2, HL, D], F32, tag="swx", name="swx")
    nc.vector.tensor_copy(
        cwx[:], rc3[:, :, 0, :].unsqueeze(2).to_broadcast([128, 2, HL, D]))
    nc.vector.tensor_copy(
        swx[:], rc3[:, :, 1, :].unsqueeze(2).to_broadcast([128, 2, HL, D]))
    qk5 = sb[:, 0:2 * HL * D].rearrange("p (g a c j) -> p g a c j",
                                        g=2 * HL, a=2, c=2)
    cw5 = cwx[:].rearrange("p t hl (a c j) -> p (t hl) a c j", a=2, c=2)
    sw5 = swx[:].rearrange("p t hl (a c j) -> p (t hl) a c j", a=2, c=2)
    rp = p["p1"].tile([128, 2 * HL, D], F32, tag="rp", name="rp")
    rp5 = rp[:].rearrange("p g (a c j) -> p g a c j", a=2, c=2)
    tmp = p["p1"].tile([128, 2 * HL, 2, 18], F32, tag="rtmp", name="rtmp")
    nc.vector.tensor_mul(rp5[:, :, :, 0, :], qk5[:, :, :, 0, :],
                         cw5[:, :, :, 0, :])
    nc.vector.tensor_mul(tmp[:], qk5[:, :, :, 1, :], sw5[:, :, :, 0, :])
    nc.vector.tensor_sub(rp5[:, :, :, 0, :], rp5[:, :, :, 0, :], tmp[:])
    nc.vector.tensor_mul(rp5[:, :, :, 1, :], qk5[:, :, :, 1, :],
                         cw5[:, :, :, 1, :])
    nc.vector.tensor_mul(tmp[:], qk5[:, :, :, 0, :], sw5[:, :, :, 1, :])
    nc.vector.tensor_add(rp5[:, :, :, 1, :], rp5[:, :, :, 1, :], tmp[:])

    # ss2[hl] = sum_d rope(q_raw)^2
    ss2 = p["p1"].tile([128, HL], F32, tag="ss2", name="ss2")
    for hl in range(HL):
        rs = rp[:, hl, :]
        nc.vector.scalar_tensor_tensor(sqd[:], rs, 1.0, rs,
                                       op0=ALU.mult, op1=ALU.mult,
                                       accum_out=ss2[:, hl:hl + 1])

    # c_q = 8*sqrt(72)*y_q*(ss2*seed(ss2)); c's precision is irrelevant
    # (cancels in softmax), only its range matters.
    y0s = p["p1"].tile([128, HL], F32, tag="y0s", name="y0s")
    m1 = p["p1"].tile([128, HL], F32, tag="m1", name="m1")
    nc.vector.tensor_scalar(y0s[:].bitcast(I32), ss2[:].bitcast(I32),
                            1, None, op0=ALU.logical_shift_right)
    nc.vector.tensor_tensor(y0s[:].bitcast(I32), p["magic"][:, 0:HL],
                            y0s[:].bitcast(I32), op=ALU.subtract)
    nc.vector.tensor_mul(m1[:], ss2[:], y0s[:])
    nc.vector.tensor_mul(m1[:], m1[:], y[:, 0:2])

    qaug = p["p1"].tile([128, HL, D + 1], F32, tag="qaug", name="qaug")
    kaug = p["p1"].tile([128, HL, D + 1], F32, tag="kaug", name="kaug")
    nc.vector.tensor_mul(qaug[:, :, D], m1[:], p["cneg"][:, 0:HL])
    nc.vector.memset(kaug[:, :, D], 1.0)

    ybq = y[:, 0:2].unsqueeze(2).to_broadcast([128, HL, D])
    ybk = y[:, 2:4].unsqueeze(2).to_broadcast([128, HL, D])
    ybv = y[:, 4:6].unsqueeze(2).to_broadcast([128, HL, D])
    nc.vector.tensor_mul(qaug[:, :, 0:D], rp[:, 0:2, :], ybq)
    nc.vector.tensor_mul(kaug[:, :, 0:D], rp[:, 2:4, :], ybk)
    nc.vector.tensor_mul(
        p["vaug"][b][:, kb, :, 0:D],
        sb[:, 2 * HL * D:3 * HL * D].rearrange("p (hl d) -> p hl d", hl=HL),
        ybv)

    # transpose q/k to feature-major; stage q for 4 blocks per DMA
    tr4 = p["trps"].tile([D + 1, 4, 128], F32, tag="tr4", name="tr4")
    for hl in range(HL):
        nc.tensor.transpose(tr4[:, hl, :], qaug[:, hl, :], p["ident"][:])
        nc.tensor.transpose(tr4[:, 2 + hl, :], kaug[:, hl, :], p["ident"][:])
    if kb < 4:
        qdst = p["qt0"][b]
    else:
        if kb % 4 == 0:
            ctx["qstg"] = p["p1"].tile([D + 1, HL, 512], F32R, tag="qstg",
                                       name="qstg")
        qdst = ctx["qstg"]
    qs = kb % 4
    nc.vector.tensor_copy(qdst[:, :, qs * 128:(qs + 1) * 128],
                          tr4[:, 0:2, :])
    nc.vector.tensor_copy(p["kt"][b][:, :, kb * 128:(kb + 1) * 128],
                          tr4[:, 2:4, :])
    if kb % 4 == 3 and kb >= 4:
        k4 = kb - 3
        nc.gpsimd.dma_start(
            p["qt_dram"][b][:, :, k4 * 128:(k4 + 4) * 128], ctx["qstg"][:])


def _emit_normalize(nc, ctx, src_ap, b, hl, qc):
    p = ctx
    rec = p["p2n"].tile([1, QC], F32, tag="rec", name="rec")
    nc.vector.reciprocal(rec[:], src_ap[96:97, :])
    bct = p["p2n"].tile([D, QC], F32, tag="bct", name="bct")
    nc.gpsimd.partition_broadcast(bct[:], rec[:])
    onrm = p["p2n"].tile([D, QC], BF16, tag="onrm", name="onrm")
    nc.vector.tensor_mul(onrm[:], src_ap[0:D, :], bct[:])
    e = b * 4 + qc // 2
    h = qc % 2
    nc.gpsimd.dma_start(p["a2a_in"][h][e, hl * D:(hl + 1) * D, :], onrm[:])


def _emit_ph2_iter(nc, ctx, b, hl, qc, defer=False):
    """Full attention for one (batch, head, 512-query chunk)."""
    p = ctx
    if qc == 0:
        qt_v = p["qt0"][b][:, hl, :]
    else:
        qsl = slice(qc * QC, (qc + 1) * QC)
        qt_t = p["p2"].tile([D + 1, QC], F32R, tag="qt", name="qt_t")
        nc.sync.dma_start(qt_t[:], p["qt_dram"][b][:, hl, qsl])
        qt_v = qt_t[:]

    pso = p["psos"].tile([97, QC], F32, tag="pso", name="pso")
    for kb in range(KBLK):
        ksl = slice(kb * 128, (kb + 1) * 128)
        pss = p["pss"].tile([128, QC], F32, tag="pss", name="pss")
        nc.tensor.matmul(pss[:], p["kt"][b][:, hl, ksl], qt_v,
                         start=True, stop=True)
        pt = p["p2"].tile([128, QC], BF16, tag="pt", name="pt")
        nc.scalar.activation(pt[:], pss[:], AF.Exp)
        nc.tensor.matmul(pso[:], p["vaug"][b][:, kb, hl, :], pt[:],
                         start=(kb == 0), stop=(kb == KBLK - 1))

    if defer:
        # stash the raw accumulator; normalize later when DVE is idle
        # (the even-half outputs aren't needed until the first AllToAll)
        st = p["stash"].tile([97, QC], BF16, tag=f"st_{b}_{hl}_{qc}",
                             name=f"st_{b}_{hl}_{qc}")
        nc.vector.tensor_copy(st[:], pso[:])
        p["deferred"].append((st, b, hl, qc))
    else:
        _emit_normalize(nc, ctx, pso[:], b, hl, qc)


def _emit_oproj_half(nc, ctx, tcn):
    p = ctx
    tsl = slice(tcn * QC, (tcn + 1) * QC)
    yt = p["p3"].tile([128, 9, QC], BF16, tag=f"yt{tcn}", name=f"yt{tcn}")
    ysrc = p["a2a_out"][tcn][:].rearrange("j r t -> (j r) t").rearrange(
        "(c p) t -> p c t", p=128)
    for fi in range(9):
        nc.sync.dma_start(yt[:, fi, :], ysrc[:, fi, :])
    for fo in range(9):
        ps3 = p["p3ps"].tile([128, QC], F32, tag="ps3", name="ps3")
        for fi in range(9):
            nc.tensor.matmul(ps3[:],
                             p["wo_sb"][:, fi, fo * 128:(fo + 1) * 128],
                             yt[:, fi, :], start=(fi == 0), stop=(fi == 8))
        ot = p["p3o"].tile([128, QC], F32, tag="ot", name="ot")
        nc.scalar.activation(ot[:], ps3[:], AF.Copy)
        nc.sync.dma_start(p["outT"][:, fo, tsl], ot[:])


def _build_nc():
    nc = bacc.Bacc("TRN2", target_bir_lowering=False, debug=False,
                   num_devices=N_CORES)

    xb = nc.dram_tensor("xb", [128, B * NBLK, 9, 128], F32R,
                        kind="ExternalInput").ap()
    rcb = nc.dram_tensor("rcb", [128, B * NBLK, 4, D], F32,
                         kind="ExternalInput").ap()
    wqkv = nc.dram_tensor("wqkv", [128, 9, 3 * HL * D], F32R,
                          kind="ExternalInput").ap()
    wo = nc.dram_tensor("wo", [128, 9, HID], BF16, kind="ExternalInput").ap()
    outT = nc.dram_tensor("outT", [128, 9, 1024], F32,
                          kind="ExternalOutput").ap()

    with tile.TileContext(nc) as tc:
        with (
            tc.tile_pool(name="persist", bufs=1) as persist,
            tc.tile_pool(name="dram", bufs=1, space="DRAM") as dram,
        ):
            ctx = {"xb": xb, "rcb": rcb, "outT": outT}

            ident = persist.tile([128, 128], F32, tag="ident")
            make_identity(nc, ident)
            ctx["ident"] = ident
            magic = persist.tile([128, 6], I32, tag="magic")
            nc.vector.memset(magic[:], MAGIC)
            ctx["magic"] = magic
            c15 = persist.tile([128, 6], F32, tag="c15")
            nc.vector.memset(c15[:], 1.5)
            ctx["c15"] = c15
            cneg = persist.tile([128, 2], F32, tag="cneg")
            nc.vector.memset(cneg[:], -BETA)
            ctx["cneg"] = cneg

            wqkv_sb = persist.tile([128, 9, 3 * HL * D], F32R, tag="wqkv")
            nc.sync.dma_start(wqkv_sb[:], wqkv)
            ctx["wqkv_sb"] = wqkv_sb

            ctx["kt"] = [persist.tile([D + 1, HL, P], F32R, tag=f"kt{b}",
                                      name=f"kt{b}") for b in range(B)]
            ctx["vaug"] = [persist.tile([128, KBLK, HL, 97], BF16,
                                        tag=f"vaug{b}", name=f"vaug{b}")
                           for b in range(B)]
            for b in range(B):
                nc.vector.memset(ctx["vaug"][b][:], 0.0)
                nc.vector.memset(ctx["vaug"][b][:, :, :, 96], 1.0)

            ctx["qt0"] = [persist.tile([D + 1, HL, QC], F32R,
                                       tag=f"qt0{b}", name=f"qt0{b}")
                          for b in range(B)]
            ctx["qt_dram"] = [dram.tile([D + 1, HL, P], F32R,
                                        name=f"qtd{b}") for b in range(B)]
            ctx["a2a_in"] = [dram.tile([N_CORES, HL * D, QC], BF16,
                                       name=f"a2ai{h}") for h in range(2)]
            ctx["a2a_out"] = [dram.tile([N_CORES, HL * D, QC], BF16,
                                        name=f"a2ao{h}") for h in range(2)]

            evens = [0, 2, 4, 6]
            odds = [1, 3, 5, 7]

            ctx["deferred"] = []
            with (
                tc.tile_pool(name="stash", bufs=1) as stash,
                tc.tile_pool(name="p2", bufs=4) as p2,
                tc.tile_pool(name="p2n", bufs=2) as p2n,
                tc.tile_pool(name="pss", bufs=4, space="PSUM") as pss,
                tc.tile_pool(name="psos", bufs=2, space="PSUM") as psos,
            ):
                ctx.update(p2=p2, p2n=p2n, pss=pss, psos=psos, stash=stash)

                # ---- phase 1 (b=0), then phase2(b=0 evens) x phase1(b=1)
                with (
                    tc.tile_pool(name="p1", bufs=3) as p1,
                    tc.tile_pool(name="p1ps", bufs=1, space="PSUM") as p1ps,
                    tc.tile_pool(name="trps", bufs=1, space="PSUM") as trps,
                ):
                    ctx.update(p1=p1, p1ps=p1ps, trps=trps)
                    for kbp in range(NBLK // 2):
                        _emit_ph1_pair(nc, ctx, 0, kbp)
                    for j, qc in enumerate(evens):
                        for hl in range(HL):
                            _emit_ph2_iter(nc, ctx, 0, hl, qc)
                            _emit_ph1_pair(nc, ctx, 1, 2 * (2 * j + hl))
                            _emit_ph1_pair(nc, ctx, 1, 2 * (2 * j + hl) + 1)

                for qc in evens:
                    for hl in range(HL):
                        _emit_ph2_iter(nc, ctx, 1, hl, qc)

                nc.gpsimd.collective_compute(
                    "AllToAll", mybir.AluOpType.bypass,
                    ins=[ctx["a2a_in"][0][:]],
                    outs=[ctx["a2a_out"][0][:]],
                    replica_groups=[list(range(N_CORES))],
                )

                with (
                    tc.tile_pool(name="p3", bufs=1) as p3,
                    tc.tile_pool(name="p3o", bufs=2) as p3o,
                    tc.tile_pool(name="p3ps", bufs=2, space="PSUM") as p3ps,
                ):
                    ctx.update(p3=p3, p3o=p3o, p3ps=p3ps)
                    wo_sb = p3.tile([128, 9, HID], BF16, tag="wo",
                                    name="wo_sb")
                    nc.sync.dma_start(wo_sb[:], wo)
                    ctx["wo_sb"] = wo_sb

                    for b in range(B):
                        for qc in odds:
                            for hl in range(HL):
                                _emit_ph2_iter(nc, ctx, b, hl, qc)

                    _emit_oproj_half(nc, ctx, 0)

                    nc.gpsimd.collective_compute(
                        "AllToAll", mybir.AluOpType.bypass,
                        ins=[ctx["a2a_in"][1][:]],
                        outs=[ctx["a2a_out"][1][:]],
                        replica_groups=[list(range(N_CORES))],
                    )

                    _emit_oproj_half(nc, ctx, 1)

    nc.compile()
    return nc


def _prep_inputs(inputs):
    hs = np.asarray(inputs["hidden_states"], dtype=np.float32)
    cos = np.asarray(inputs["cos"], dtype=np.float32)
    sin = np.asarray(inputs["sin"], dtype=np.float32)
    Wq = np.asarray(inputs["Wq"], dtype=np.float32)
    Wk = np.asarray(inputs["Wk"], dtype=np.float32)
    Wv = np.asarray(inputs["Wv"], dtype=np.float32)
    Wo = np.asarray(inputs["Wo"], dtype=np.float32)
    qw = np.asarray(inputs["q_norm_w"], dtype=np.float32)
    kw = np.asarray(inputs["k_norm_w"], dtype=np.float32)

    # x blocked: xb[p, blk, c, ti] = x[t=blk*128+ti, h=c*128+p]
    xb = np.ascontiguousarray(
        hs.reshape(TB // 128, 128, 9, 128).transpose(3, 0, 2, 1))

    # rope coefficients per token: rows [cwq, swq, cwk, swk]
    partner = np.empty(D, np.int64)
    for a in range(2):
        base = a * 36
        partner[base:base + 18] = np.arange(base + 18, base + 36)
        partner[base + 18:base + 36] = np.arange(base, base + 18)
    cs = cos.reshape(TB, D)
    sn = sin.reshape(TB, D)
    ropec = np.stack([cs * qw[None, :], sn * qw[partner][None, :],
                      cs * kw[None, :], sn * kw[partner][None, :]],
                     axis=1).astype(np.float32)
    rcb = np.ascontiguousarray(
        ropec.reshape(TB // 128, 128, 4, D).transpose(1, 0, 2, 3))

    wob = np.ascontiguousarray(
        Wo.reshape(9, 128, HID).transpose(1, 0, 2)).astype(
            ml_dtypes.bfloat16)

    in_maps = []
    for c in range(N_CORES):
        colsl = slice(c * HL * D, (c + 1) * HL * D)
        wqkv = np.concatenate([Wq[:, colsl], Wk[:, colsl], Wv[:, colsl]],
                              axis=1)
        wqkv = np.ascontiguousarray(
            wqkv.reshape(9, 128, 3 * HL * D).transpose(1, 0, 2))
        in_maps.append({
            "xb": xb,
            "rcb": rcb,
            "wqkv": wqkv,
            "wo": wob,
        })
    return in_maps


def kernel(**inputs):
    global _CACHED_NC
    if _CACHED_NC is None:
        _CACHED_NC = _build_nc()
    nc = _CACHED_NC
    in_maps = _prep_inputs(inputs)
    trace = bool(int(os.environ.get("KERNEL_TRACE", "0")))
    res = run_bass_kernel_spmd(nc, in_maps, core_ids=list(range(N_CORES)),
                               trace=trace)
    kernel.last_results = res
    out = np.empty((B, P, HID), dtype=np.float32)
    for c in range(N_CORES):
        b, qtr = c // 4, c % 4
        r = res.results[c]["outT"]  # [128, 9, 1024]
        out[b, qtr * 1024:(qtr + 1) * 1024, :] = \
            np.asarray(r).transpose(2, 1, 0).reshape(1024, HID)
    return out
